# revision 23
# baseline (speedup 1.0000x reference)
"""Trainium2 Bass kernel for a pre-LN multi-head attention block.

Model (per batch b): LayerNorm(x) -> QKV -> 16-head attention (dh=64) ->
output projection + bias.

Sharding over 8 NeuronCores: core c handles batch b = c//2 and query
seq-half s = c%2 (all 16 heads, 1024 query rows, full 2048 keys).  K/V
projections are duplicated across the pair, but outputs are disjoint row
slices, so unsharding is a pure host-side concat (no collectives).

The same NEFF runs on every core: the host hands odd cores x with its two
seq halves swapped, so "my queries" are always rows 0:1023 of the local
view.  Attention results are invariant to key/value ordering (softmax sum
and PV sum are permutation-invariant), so the swapped key order on odd
cores changes nothing.

Device-side layout notes:
 - Activations are kept transposed (feature dim on partitions): every
   matmul contracts over the partition axis.
 - Scores are computed directly as S^T [nk, nq]; softmax needs no max
   subtraction (scores ~ N(0,1)), so exp is one ScalarE pass and the
   denominator rides along as a ones-column in the PV matmul (M=65).
 - ln_gamma/ln_beta are folded into the QKV weights host-side; b_out is
   added host-side.
 - QKV weight tiles stream from DRAM per output tile; q/k/V_ext tiles are
   transient, produced per head-pair right before that pair's attention,
   which keeps TensorE densely busy (HAM stays at full clock).
"""

import numpy as np
from ml_dtypes import bfloat16

B, N, D = 4, 2048, 1024
HEADS, DH = 16, 64
SCALE = DH ** -0.5
NCORES = 8
NQ = N // 2                 # 1024 query rows per core
EPS = 1e-5
NT = N // 128               # 16 sequence tiles (LN)
KD = D // 128               # 8 feature tiles
NKT = N // 128              # 16 key tiles
NOB = 3 * D // 128          # 24 qkv output row-tiles (q:0-7, k:8-15, v:16-23)

_cache = {}


def _build():
    import concourse.bass as bass
    import concourse.mybir as mybir
    import concourse.bacc as bacc
    import concourse.tile as tile
    from concourse.masks import make_identity

    f32 = mybir.dt.float32
    bf16 = mybir.dt.bfloat16
    AX = mybir.AxisListType
    ALU = mybir.AluOpType
    ACTF = mybir.ActivationFunctionType

    nc = bacc.Bacc(
        "TRN2",
        target_bir_lowering=False,
        debug=False,
        enable_asserts=True,
        num_devices=NCORES,
    )

    x_d = nc.dram_tensor("x", [N, D], f32, kind="ExternalInput").ap()
    wq_d = nc.dram_tensor("wqkvT", [D, 3 * D], bf16, kind="ExternalInput").ap()
    bias_d = nc.dram_tensor("qkv_bias", [128, NOB], f32, kind="ExternalInput").ap()
    wo_d = nc.dram_tensor("woutT", [D, D], bf16, kind="ExternalInput").ap()
    out_d = nc.dram_tensor("out", [NQ, D], f32, kind="ExternalOutput").ap()

    with tile.TileContext(nc) as tc:
        with (
            tc.tile_pool(name="persist", bufs=1) as P,
            tc.tile_pool(name="ppool", bufs=1, space="PSUM") as PS,
            tc.tile_pool(name="trans", bufs=1) as T,
        ):
            ident = P.tile([128, 128], bf16, name="ident", tag="ident")
            make_identity(nc, ident)
            eps_t = P.tile([128, 1], f32, name="eps_t", tag="eps_t")
            nc.vector.memset(eps_t, EPS)

            bias_sb = P.tile([128, NOB], f32, name="bias_sb", tag="bias_sb")
            nc.sync.dma_start(bias_sb, bias_d)

            wo_sb = []
            for k in range(KD):
                t = P.tile([128, D], bf16, name=f"wo{k}", tag=f"wo{k}")
                nc.sync.dma_start(t, wo_d[k * 128:(k + 1) * 128, :])
                wo_sb.append(t)

            # xnT: transposed normalized activations [d, n] as [128, KD*N]
            xnT = P.tile([128, KD * N], bf16, name="xnT", tag="xnT")
            xnT3 = xnT.rearrange("p (k n) -> p k n", k=KD)
            # normalized attention outputs, transposed: [1024 hd, 1024 nq]
            onormT = []
            for k in range(KD):
                onormT.append(
                    P.tile([128, NQ], bf16, name=f"onormT{k}", tag=f"onormT{k}")
                )

            sq_scr = T.tile([128, D], f32, name="sq_scr", tag="sq", bufs=1)

            # ---- Phase A: LayerNorm + transpose, pipelined over seq tiles
            for nt in range(NT):
                x_t = T.tile([128, D], f32, name=f"x{nt}", tag="x", bufs=3)
                nc.sync.dma_start(x_t, x_d[nt * 128:(nt + 1) * 128, :])
                ssum = T.tile([128, 1], f32, name=f"ss{nt}", tag="ss", bufs=3)
                nc.scalar.activation(sq_scr, x_t, ACTF.Copy, accum_out=ssum)
                mean = T.tile([128, 1], f32, name=f"mn{nt}", tag="mn", bufs=3)
                nc.scalar.mul(mean, ssum, 1.0 / D)
                xc = T.tile([128, D], f32, name=f"xc{nt}", tag="xc", bufs=3)
                nc.vector.tensor_scalar_sub(xc, x_t, mean)
                var = T.tile([128, 1], f32, name=f"vr{nt}", tag="vr", bufs=3)
                nc.scalar.activation(sq_scr, xc, ACTF.Square, accum_out=var)
                std = T.tile([128, 1], f32, name=f"st{nt}", tag="st", bufs=3)
                nc.scalar.activation(std, var, ACTF.Sqrt, bias=eps_t, scale=1.0 / D)
                rstd = T.tile([128, 1], f32, name=f"rs{nt}", tag="rs", bufs=3)
                nc.vector.reciprocal(rstd, std)
                xhat = T.tile([128, D], bf16, name=f"xh{nt}", tag="xh", bufs=3)
                nc.vector.tensor_scalar_mul(xhat, xc, rstd)
                for g2 in range(2):
                    tp = PS.tile(
                        [128, 512], bf16, name=f"tp{nt}_{g2}",
                        tag="work", bufs=2
                    )
                    for j in range(4):
                        kd = g2 * 4 + j
                        nc.tensor.transpose(
                            tp[:, j * 128:(j + 1) * 128],
                            xhat[:, kd * 128:(kd + 1) * 128],
                            ident,
                        )
                    dest = xnT3[:, g2 * 4:(g2 + 1) * 4, nt * 128:(nt + 1) * 128]
                    src = tp.rearrange("p (k n) -> p k n", k=4)
                    if (nt + g2) % 2 == 0:
                        nc.vector.tensor_copy(dest, src)
                    else:
                        nc.scalar.copy(dest, src)

            # QKV projection work for pair j is packaged as a list of
            # emission closures so it can be interleaved into pair j-1's
            # attention loop — the scheduler then overlaps next-pair QKV
            # (K=128 matmuls, which also keep the PE HAM at full clock)
            # with the current pair's ACT-paced softmax.
            def qkv_emitters(j, store):
                ems = []
                for ob, ncols, key in ((j, NQ, "qT"), (8 + j, N, "kT")):
                    def alloc(j=j, ob=ob, ncols=ncols, key=key):
                        wts = []
                        store[(key, "w")] = wts
                        for k in range(KD):
                            wt = T.tile(
                                [128, 128], bf16, name=f"w{key}{j}_{k}",
                                tag=f"wqs{k}", bufs=3,
                            )
                            nc.sync.dma_start(
                                wt,
                                wq_d[k * 128:(k + 1) * 128,
                                     ob * 128:(ob + 1) * 128],
                            )
                            wts.append(wt)
                        store[key] = T.tile(
                            [128, ncols], bf16, name=f"t{key}{j}", tag=key,
                            bufs=3,
                        )
                    ems.append(alloc)
                    for c in range(ncols // 512):
                        def chunk(j=j, c=c, ob=ob, key=key):
                            qp = PS.tile(
                                [128, 512], f32, name=f"qp{key}{j}_{c}",
                                tag="work", bufs=2,
                            )
                            wts = store[(key, "w")]
                            for k in range(KD):
                                nc.tensor.matmul(
                                    qp,
                                    lhsT=wts[k],
                                    rhs=xnT3[:, k, c * 512:(c + 1) * 512],
                                    start=(k == 0),
                                    stop=(k == KD - 1),
                                )
                            dcol = store[key][:, c * 512:(c + 1) * 512]
                            nc.vector.tensor_scalar_add(
                                dcol, qp, bias_sb[:, ob:ob + 1]
                            )
                        ems.append(chunk)
                # V in natural [nk, dh] layout straight from the
                # projection (xnT tile stationary, weight tile moving) —
                # no PE transposes.  Pair layout [128, kt, {V0|1|V1|1}]:
                # per-head lhsT slice [128, 65] includes its ones column.
                def valloc(j=j):
                    wts = []
                    store[("v", "w")] = wts
                    for k in range(KD):
                        wt = T.tile(
                            [128, 128], bf16, name=f"wv{j}_{k}",
                            tag=f"wvs{k}", bufs=2,
                        )
                        nc.sync.dma_start(
                            wt,
                            wq_d[k * 128:(k + 1) * 128,
                                 2048 + j * 128:2048 + (j + 1) * 128],
                        )
                        wts.append(wt)
                    ve = T.tile(
                        [128, NKT * 130], bf16, name=f"vx{j}", tag="vext",
                        bufs=2,
                    )
                    nc.vector.memset(ve, 1.0)
                    store["ve4"] = ve.rearrange("p (k t e) -> p k t e",
                                                t=2, e=65)
                ems.append(valloc)
                for kt in range(NKT):
                    def vjob(j=j, kt=kt):
                        wts = store[("v", "w")]
                        vp = PS.tile(
                            [128, 128], f32, name=f"vp{j}_{kt}",
                            tag="work", bufs=2,
                        )
                        for k in range(KD):
                            nc.tensor.matmul(
                                vp,
                                lhsT=xnT3[:, k, kt * 128:(kt + 1) * 128],
                                rhs=wts[k],
                                start=(k == 0),
                                stop=(k == KD - 1),
                            )
                        dest = store["ve4"][:, kt, :, 0:64]
                        nc.vector.tensor_copy(
                            dest, vp.rearrange("p (t e) -> p t e", e=64)
                        )
                    ems.append(vjob)
                return ems

            # ---- Phases B+C: per head pair, attention row-packed via
            # tile_position so K stays covered (K<128 matmuls don't count
            # as PE-busy for the HAM clock gate).
            stores = [dict() for _ in range(KD + 1)]
            for e in qkv_emitters(0, stores[0]):
                e()
            for j in range(KD):
                st = stores[j]
                qT_j, kT_j, ve4 = st["qT"], st["kT"], st["ve4"]
                pend = qkv_emitters(j + 1, stores[j + 1]) if j + 1 < KD else []
                pi = 0
                for blk in range(2):
                    b0 = blk * 512
                    opss = [
                        PS.tile([65, 512], f32, name=f"ops{2*j}_{blk}",
                                tag="acc0", bufs=1),
                        PS.tile([65, 512], f32, name=f"ops{2*j+1}_{blk}",
                                tag="acc1", bufs=1),
                    ]
                    for kt in range(NKT):
                        sps = PS.tile(
                            [128, 1024], f32, name=f"s{j}_{blk}_{kt}",
                            tag="spair", bufs=2,
                        )
                        for h2 in range(2):
                            p0 = h2 * 64
                            nc.tensor.matmul(
                                sps[:, h2 * 512:(h2 + 1) * 512],
                                lhsT=kT_j[p0:p0 + 64, kt * 128:(kt + 1) * 128],
                                rhs=qT_j[p0:p0 + 64, b0:b0 + 512],
                                start=True,
                                stop=True,
                                tile_position=(p0, 0),
                            )
                        pt = T.tile(
                            [128, 1024], bf16, name=f"pt{j}_{blk}_{kt}",
                            tag="pt", bufs=6,
                        )
                        nc.scalar.activation(pt, sps, ACTF.Exp, scale=SCALE)
                        for h2 in range(2):
                            nc.tensor.matmul(
                                opss[h2],
                                lhsT=ve4[:, kt, h2, :],
                                rhs=pt[:, h2 * 512:(h2 + 1) * 512],
                                start=(kt == 0),
                                stop=(kt == NKT - 1),
                            )
                        # interleave next-pair QKV emission across kt slots
                        it = blk * NKT + kt + 1
                        while pi < len(pend) and pi * 2 * NKT < len(pend) * it:
                            pend[pi]()
                            pi += 1
                    # evict accumulators to SBUF fast (frees the psum bank),
                    # then normalize off the critical path
                    for h2 in range(2):
                        h = 2 * j + h2
                        p0 = h2 * 64
                        oc = T.tile([65, 512], f32, name=f"oc{h}_{blk}",
                                    tag="oc", bufs=3)
                        nc.vector.tensor_copy(oc, opss[h2])
                        rl = T.tile([1, 512], f32, name=f"rl{h}_{blk}",
                                    tag="rl", bufs=2)
                        nc.vector.reciprocal(rl, oc[64:65, :])
                        rlb = T.tile([64, 512], f32, name=f"rlb{h}_{blk}",
                                     tag="rlb", bufs=2)
                        nc.gpsimd.partition_broadcast(rlb, rl, channels=64)
                        nc.vector.tensor_mul(
                            onormT[h // 2][p0:p0 + 64, b0:b0 + 512],
                            oc[0:64, :],
                            rlb,
                        )
                while pi < len(pend):
                    pend[pi]()
                    pi += 1

            # ---- Phase D: output projection [1024 nq, 1024 dm]
            for nt in range(NQ // 128):
                po = T.tile([128, D], f32, name=f"po{nt}", tag="po", bufs=2)
                for c in range(2):
                    pp = PS.tile(
                        [128, 512], f32, name=f"pp{nt}_{c}",
                        tag="work", bufs=2
                    )
                    for kq in range(KD):
                        nc.tensor.matmul(
                            pp,
                            lhsT=onormT[kq][:, nt * 128:(nt + 1) * 128],
                            rhs=wo_sb[kq][:, c * 512:(c + 1) * 512],
                            start=(kq == 0),
                            stop=(kq == KD - 1),
                        )
                    if c == 0:
                        nc.scalar.copy(po[:, c * 512:(c + 1) * 512], pp)
                    else:
                        nc.vector.tensor_copy(po[:, c * 512:(c + 1) * 512], pp)
                nc.sync.dma_start(out_d[nt * 128:(nt + 1) * 128, :], po)

    nc.compile()
    return nc


def _shard_inputs(x, ln_gamma, ln_beta, w_qkv, w_out):
    w_eff = (w_qkv * ln_gamma[None, :]).astype(np.float32)
    wqkvT = np.ascontiguousarray(w_eff.T).astype(bfloat16)          # [1024, 3072]
    bias = (w_qkv.astype(np.float64) @ ln_beta.astype(np.float64))
    bias_2d = np.ascontiguousarray(
        bias.reshape(NOB, 128).T
    ).astype(np.float32)                                            # [128, 24]
    woutT = np.ascontiguousarray(w_out.T).astype(bfloat16)          # [1024, 1024]
    in_maps = []
    for c in range(NCORES):
        b, s = c // 2, c % 2
        xb = np.asarray(x[b], dtype=np.float32)
        if s == 1:
            xb = np.concatenate([xb[NQ:], xb[:NQ]], axis=0)
        in_maps.append({
            "x": np.ascontiguousarray(xb),
            "wqkvT": wqkvT,
            "qkv_bias": bias_2d,
            "woutT": woutT,
        })
    return in_maps


def kernel(x, ln_gamma, ln_beta, w_qkv, w_out, b_out, _trace=False):
    from concourse import bass_utils

    x = np.asarray(x, dtype=np.float32)
    ln_gamma = np.asarray(ln_gamma, dtype=np.float32)
    ln_beta = np.asarray(ln_beta, dtype=np.float32)
    w_qkv = np.asarray(w_qkv, dtype=np.float32)
    w_out = np.asarray(w_out, dtype=np.float32)
    b_out = np.asarray(b_out, dtype=np.float32)

    if "nc" not in _cache:
        _cache["nc"] = _build()
    nc = _cache["nc"]

    in_maps = _shard_inputs(x, ln_gamma, ln_beta, w_qkv, w_out)
    res = bass_utils.run_bass_kernel_spmd(
        nc, in_maps, core_ids=list(range(NCORES)), trace=_trace
    )
    out = np.empty((B, N, D), dtype=np.float32)
    for b in range(B):
        out[b, :NQ] = np.asarray(res.results[2 * b]["out"])
        out[b, NQ:] = np.asarray(res.results[2 * b + 1]["out"])
    out += b_out[None, None, :]
    _cache["last_result"] = res
    return out


# revision 24
# speedup vs baseline: 1.1148x; 1.1148x over previous
"""Trainium2 Bass kernel for a pre-LN multi-head attention block.

Model (per batch b): LayerNorm(x) -> QKV -> 16-head attention (dh=64) ->
output projection + bias.

Sharding over 8 NeuronCores: core c handles batch b = c//2 and query
seq-half s = c%2 (all 16 heads, 1024 query rows, full 2048 keys).  K/V
projections are duplicated across the pair, but outputs are disjoint row
slices, so unsharding is a pure host-side concat (no collectives).

The same NEFF runs on every core: the host hands odd cores x with its two
seq halves swapped, so "my queries" are always rows 0:1023 of the local
view.  Attention results are invariant to key/value ordering (softmax sum
and PV sum are permutation-invariant), so the swapped key order on odd
cores changes nothing.

Device-side layout notes:
 - Activations are kept transposed (feature dim on partitions): every
   matmul contracts over the partition axis.
 - Scores are computed directly as S^T [nk, nq]; softmax needs no max
   subtraction (scores ~ N(0,1)), so exp is one ScalarE pass and the
   denominator rides along as a ones-column in the PV matmul (M=65).
 - ln_gamma/ln_beta are folded into the QKV weights host-side; b_out is
   added host-side.
 - QKV weight tiles stream from DRAM per output tile; q/k/V_ext tiles are
   transient, produced per head-pair right before that pair's attention,
   which keeps TensorE densely busy (HAM stays at full clock).
"""

import numpy as np
from ml_dtypes import bfloat16

B, N, D = 4, 2048, 1024
HEADS, DH = 16, 64
SCALE = DH ** -0.5
NCORES = 8
NQ = N // 2                 # 1024 query rows per core
EPS = 1e-5
NT = N // 128               # 16 sequence tiles (LN)
KD = D // 128               # 8 feature tiles
NKT = N // 128              # 16 key tiles
NOB = 3 * D // 128          # 24 qkv output row-tiles (q:0-7, k:8-15, v:16-23)

_cache = {}


def _build():
    import concourse.bass as bass
    import concourse.mybir as mybir
    import concourse.bacc as bacc
    import concourse.tile as tile
    from concourse.masks import make_identity

    f32 = mybir.dt.float32
    bf16 = mybir.dt.bfloat16
    AX = mybir.AxisListType
    ALU = mybir.AluOpType
    ACTF = mybir.ActivationFunctionType

    nc = bacc.Bacc(
        "TRN2",
        target_bir_lowering=False,
        debug=False,
        enable_asserts=True,
        num_devices=NCORES,
    )

    x_d = nc.dram_tensor("x", [N, D], f32, kind="ExternalInput").ap()
    wq_d = nc.dram_tensor("wqkvT", [D, 3 * D], bf16, kind="ExternalInput").ap()
    bias_d = nc.dram_tensor("qkv_bias", [128, NOB], f32, kind="ExternalInput").ap()
    wo_d = nc.dram_tensor("woutT", [D, D], bf16, kind="ExternalInput").ap()
    out_d = nc.dram_tensor("out", [NQ, D], f32, kind="ExternalOutput").ap()

    with tile.TileContext(nc) as tc:
        with (
            tc.tile_pool(name="persist", bufs=1) as P,
            tc.tile_pool(name="ppool", bufs=1, space="PSUM") as PS,
            tc.tile_pool(name="trans", bufs=1) as T,
        ):
            ident = P.tile([128, 128], bf16, name="ident", tag="ident")
            make_identity(nc, ident)
            eps_t = P.tile([128, 1], f32, name="eps_t", tag="eps_t")
            nc.vector.memset(eps_t, EPS)

            bias_sb = P.tile([128, NOB], f32, name="bias_sb", tag="bias_sb")
            nc.sync.dma_start(bias_sb, bias_d)

            wo_sb = []
            for k in range(KD):
                t = P.tile([128, D], bf16, name=f"wo{k}", tag=f"wo{k}")
                nc.sync.dma_start(t, wo_d[k * 128:(k + 1) * 128, :])
                wo_sb.append(t)

            # xnT: transposed normalized activations [d, n] as [128, KD*N]
            xnT = P.tile([128, KD * N], bf16, name="xnT", tag="xnT")
            xnT3 = xnT.rearrange("p (k n) -> p k n", k=KD)
            # normalized attention outputs, transposed: [1024 hd, 1024 nq]
            onormT = []
            for k in range(KD):
                onormT.append(
                    P.tile([128, NQ], bf16, name=f"onormT{k}", tag=f"onormT{k}")
                )

            sq_scr = T.tile([128, D], f32, name="sq_scr", tag="sq", bufs=1)

            # ---- Phase A: LayerNorm + transpose, pipelined over seq tiles
            for nt in range(NT):
                x_t = T.tile([128, D], f32, name=f"x{nt}", tag="x", bufs=3)
                nc.sync.dma_start(x_t, x_d[nt * 128:(nt + 1) * 128, :])
                ssum = T.tile([128, 1], f32, name=f"ss{nt}", tag="ss", bufs=3)
                nc.scalar.activation(sq_scr, x_t, ACTF.Copy, accum_out=ssum)
                mean = T.tile([128, 1], f32, name=f"mn{nt}", tag="mn", bufs=3)
                nc.scalar.mul(mean, ssum, 1.0 / D)
                xc = T.tile([128, D], f32, name=f"xc{nt}", tag="xc", bufs=3)
                nc.vector.tensor_scalar_sub(xc, x_t, mean)
                var = T.tile([128, 1], f32, name=f"vr{nt}", tag="vr", bufs=3)
                nc.scalar.activation(sq_scr, xc, ACTF.Square, accum_out=var)
                std = T.tile([128, 1], f32, name=f"st{nt}", tag="st", bufs=3)
                nc.scalar.activation(std, var, ACTF.Sqrt, bias=eps_t, scale=1.0 / D)
                rstd = T.tile([128, 1], f32, name=f"rs{nt}", tag="rs", bufs=3)
                nc.vector.reciprocal(rstd, std)
                xhat = T.tile([128, D], bf16, name=f"xh{nt}", tag="xh", bufs=3)
                nc.vector.tensor_scalar_mul(xhat, xc, rstd)
                for g2 in range(2):
                    tp = PS.tile(
                        [128, 512], bf16, name=f"tp{nt}_{g2}",
                        tag="work", bufs=2
                    )
                    for j in range(4):
                        kd = g2 * 4 + j
                        nc.tensor.transpose(
                            tp[:, j * 128:(j + 1) * 128],
                            xhat[:, kd * 128:(kd + 1) * 128],
                            ident,
                        )
                    dest = xnT3[:, g2 * 4:(g2 + 1) * 4, nt * 128:(nt + 1) * 128]
                    src = tp.rearrange("p (k n) -> p k n", k=4)
                    if (nt + g2) % 2 == 0:
                        nc.vector.tensor_copy(dest, src)
                    else:
                        nc.scalar.copy(dest, src)

            # QKV projection work for pair j is packaged as a list of
            # emission closures so it can be interleaved into pair j-1's
            # attention loop — the scheduler then overlaps next-pair QKV
            # (K=128 matmuls, which also keep the PE HAM at full clock)
            # with the current pair's ACT-paced softmax.
            def qkv_emitters(j, store):
                ems = []
                for ob, ncols, key in ((j, NQ, "qT"), (8 + j, N, "kT"),
                                       (16 + j, N, "vT")):
                    def alloc(j=j, ob=ob, ncols=ncols, key=key):
                        wts = []
                        store[(key, "w")] = wts
                        for k in range(KD):
                            wt = T.tile(
                                [128, 128], bf16, name=f"w{key}{j}_{k}",
                                tag=f"wqs{k}", bufs=3,
                            )
                            nc.sync.dma_start(
                                wt,
                                wq_d[k * 128:(k + 1) * 128,
                                     ob * 128:(ob + 1) * 128],
                            )
                            wts.append(wt)
                        store[key] = T.tile(
                            [128, ncols], bf16, name=f"t{key}{j}", tag=key,
                            bufs=3,
                        )
                    ems.append(alloc)
                    for c in range(ncols // 512):
                        def chunk(j=j, c=c, ob=ob, key=key):
                            qp = PS.tile(
                                [128, 512], f32, name=f"qp{key}{j}_{c}",
                                tag="work", bufs=2,
                            )
                            wts = store[(key, "w")]
                            for k in range(KD):
                                nc.tensor.matmul(
                                    qp,
                                    lhsT=wts[k],
                                    rhs=xnT3[:, k, c * 512:(c + 1) * 512],
                                    start=(k == 0),
                                    stop=(k == KD - 1),
                                )
                            dcol = store[key][:, c * 512:(c + 1) * 512]
                            nc.vector.tensor_scalar_add(
                                dcol, qp, bias_sb[:, ob:ob + 1]
                            )
                        ems.append(chunk)
                # V_ext for the two heads: [nk, dh | ones] blocks per key tile
                for h2 in range(2):
                    def valloc(j=j, h2=h2):
                        ve = T.tile(
                            [128, NKT * 65], bf16, name=f"vx{j}_{h2}",
                            tag="vext", bufs=4,
                        )
                        nc.vector.memset(ve, 1.0)
                        store[("ve", h2)] = ve.rearrange("p (k e) -> p k e", e=65)
                    ems.append(valloc)
                    for g2 in range(2):
                        def vtr(j=j, h2=h2, g2=g2):
                            p0 = h2 * 64
                            id64 = ident[p0:p0 + 64, p0:p0 + 64]
                            vT_j = store["vT"]
                            ve3 = store[("ve", h2)]
                            tp = PS.tile(
                                [128, 512], bf16, name=f"vt{j}_{h2}_{g2}",
                                tag="work", bufs=2,
                            )
                            for i8 in range(8):
                                kt = g2 * 8 + i8
                                nc.tensor.transpose(
                                    tp[:, i8 * 64:(i8 + 1) * 64],
                                    vT_j[p0:p0 + 64, kt * 128:(kt + 1) * 128],
                                    id64,
                                )
                            dest = ve3[:, g2 * 8:(g2 + 1) * 8, 0:64]
                            src = tp.rearrange("p (k e) -> p k e", e=64)
                            nc.vector.tensor_copy(dest, src)
                        ems.append(vtr)
                return ems

            # ---- Phases B+C: per head pair, attention row-packed via
            # tile_position so K stays covered (K<128 matmuls don't count
            # as PE-busy for the HAM clock gate).
            stores = [dict() for _ in range(KD + 1)]
            for e in qkv_emitters(0, stores[0]):
                e()
            for j in range(KD):
                st = stores[j]
                qT_j, kT_j = st["qT"], st["kT"]
                ve3s = [st[("ve", 0)], st[("ve", 1)]]
                pend = qkv_emitters(j + 1, stores[j + 1]) if j + 1 < KD else []
                pi = 0
                for blk in range(2):
                    b0 = blk * 512
                    opss = [
                        PS.tile([65, 512], f32, name=f"ops{2*j}_{blk}",
                                tag="acc0", bufs=1),
                        PS.tile([65, 512], f32, name=f"ops{2*j+1}_{blk}",
                                tag="acc1", bufs=1),
                    ]
                    for kt in range(NKT):
                        sps = PS.tile(
                            [128, 1024], f32, name=f"s{j}_{blk}_{kt}",
                            tag="spair", bufs=2,
                        )
                        for h2 in range(2):
                            p0 = h2 * 64
                            nc.tensor.matmul(
                                sps[:, h2 * 512:(h2 + 1) * 512],
                                lhsT=kT_j[p0:p0 + 64, kt * 128:(kt + 1) * 128],
                                rhs=qT_j[p0:p0 + 64, b0:b0 + 512],
                                start=True,
                                stop=True,
                                tile_position=(p0, 0),
                            )
                        pt = T.tile(
                            [128, 1024], bf16, name=f"pt{j}_{blk}_{kt}",
                            tag="pt", bufs=6,
                        )
                        nc.scalar.activation(pt, sps, ACTF.Exp, scale=SCALE)
                        for h2 in range(2):
                            nc.tensor.matmul(
                                opss[h2],
                                lhsT=ve3s[h2][:, kt, :],
                                rhs=pt[:, h2 * 512:(h2 + 1) * 512],
                                start=(kt == 0),
                                stop=(kt == NKT - 1),
                            )
                        # interleave next-pair QKV emission across kt slots
                        it = blk * NKT + kt + 1
                        while pi < len(pend) and pi * NKT < len(pend) * it:
                            pend[pi]()
                            pi += 1
                    # evict accumulators to SBUF fast (frees the psum bank),
                    # then normalize off the critical path
                    for h2 in range(2):
                        h = 2 * j + h2
                        p0 = h2 * 64
                        oc = T.tile([65, 512], f32, name=f"oc{h}_{blk}",
                                    tag="oc", bufs=3)
                        nc.vector.tensor_copy(oc, opss[h2])
                        rl = T.tile([1, 512], f32, name=f"rl{h}_{blk}",
                                    tag="rl", bufs=2)
                        nc.vector.reciprocal(rl, oc[64:65, :])
                        rlb = T.tile([64, 512], f32, name=f"rlb{h}_{blk}",
                                     tag="rlb", bufs=2)
                        nc.gpsimd.partition_broadcast(rlb, rl, channels=64)
                        nc.vector.tensor_mul(
                            onormT[h // 2][p0:p0 + 64, b0:b0 + 512],
                            oc[0:64, :],
                            rlb,
                        )
                while pi < len(pend):
                    pend[pi]()
                    pi += 1

            # ---- Phase D: output projection [1024 nq, 1024 dm]
            for nt in range(NQ // 128):
                po = T.tile([128, D], f32, name=f"po{nt}", tag="po", bufs=2)
                for c in range(2):
                    pp = PS.tile(
                        [128, 512], f32, name=f"pp{nt}_{c}",
                        tag="work", bufs=2
                    )
                    for kq in range(KD):
                        nc.tensor.matmul(
                            pp,
                            lhsT=onormT[kq][:, nt * 128:(nt + 1) * 128],
                            rhs=wo_sb[kq][:, c * 512:(c + 1) * 512],
                            start=(kq == 0),
                            stop=(kq == KD - 1),
                        )
                    if c == 0:
                        nc.scalar.copy(po[:, c * 512:(c + 1) * 512], pp)
                    else:
                        nc.vector.tensor_copy(po[:, c * 512:(c + 1) * 512], pp)
                nc.sync.dma_start(out_d[nt * 128:(nt + 1) * 128, :], po)

    nc.compile()
    return nc


def _shard_inputs(x, ln_gamma, ln_beta, w_qkv, w_out):
    w_eff = (w_qkv * ln_gamma[None, :]).astype(np.float32)
    wqkvT = np.ascontiguousarray(w_eff.T).astype(bfloat16)          # [1024, 3072]
    bias = (w_qkv.astype(np.float64) @ ln_beta.astype(np.float64))
    bias_2d = np.ascontiguousarray(
        bias.reshape(NOB, 128).T
    ).astype(np.float32)                                            # [128, 24]
    woutT = np.ascontiguousarray(w_out.T).astype(bfloat16)          # [1024, 1024]
    in_maps = []
    for c in range(NCORES):
        b, s = c // 2, c % 2
        xb = np.asarray(x[b], dtype=np.float32)
        if s == 1:
            xb = np.concatenate([xb[NQ:], xb[:NQ]], axis=0)
        in_maps.append({
            "x": np.ascontiguousarray(xb),
            "wqkvT": wqkvT,
            "qkv_bias": bias_2d,
            "woutT": woutT,
        })
    return in_maps


def kernel(x, ln_gamma, ln_beta, w_qkv, w_out, b_out, _trace=False):
    from concourse import bass_utils

    x = np.asarray(x, dtype=np.float32)
    ln_gamma = np.asarray(ln_gamma, dtype=np.float32)
    ln_beta = np.asarray(ln_beta, dtype=np.float32)
    w_qkv = np.asarray(w_qkv, dtype=np.float32)
    w_out = np.asarray(w_out, dtype=np.float32)
    b_out = np.asarray(b_out, dtype=np.float32)

    if "nc" not in _cache:
        _cache["nc"] = _build()
    nc = _cache["nc"]

    in_maps = _shard_inputs(x, ln_gamma, ln_beta, w_qkv, w_out)
    res = bass_utils.run_bass_kernel_spmd(
        nc, in_maps, core_ids=list(range(NCORES)), trace=_trace
    )
    out = np.empty((B, N, D), dtype=np.float32)
    for b in range(B):
        out[b, :NQ] = np.asarray(res.results[2 * b]["out"])
        out[b, NQ:] = np.asarray(res.results[2 * b + 1]["out"])
    out += b_out[None, None, :]
    _cache["last_result"] = res
    return out
